# revision 1
# baseline (speedup 1.0000x reference)
"""nn_CGBlock Trainium2 kernel: grouped channel softmax-attention branch +
grouped top-k branch, softmax-mixed, for x [16, 256, 128, 128] f32.

Data-parallel over batch: 8 NeuronCores x 2 batches each.

Raw-Bass implementation (explicit semaphores; the Tile scheduler emits
multi-wait instructions that this walrus build cannot encode - it allows
only one sync wait per instruction, so every cross-engine dependency here
is a standalone single-wait `wait_ge`).

Per-core pipeline over h-blocks of HBLK=4 rows (one tile = one h row =
128 pixels on SBUF partitions after transpose):
  SP   : channel-major HBM loads x0/x1 [128ch_half, 512 pix], stores.
  ACT  : e = exp(x); all PSUM->SBUF copies (x^T, s/num, z^T).
  Pool : xe = x*e; y = num/s (GPSIMD cannot touch PSUM, hence the sn copy).
  PE   : per-tile transposes x -> pixel-major; tiny matmuls vs constant
         masks give per-(pixel,group) s = sum_c e, num = sum_c x*e*w1;
         transpose z = [y | top8] back to z-major; delta = W2eff @ z with
         both second 1x1 convs, top_w1, and softmax(r) mixing folded in.
  DVE  : hardware top-8 `max` per (pixel,group) 32-channel window (exact
         descending-sort semantics incl. duplicates); out = x + delta.

Software pipelining: block i's tail (z transpose, delta, adds, stores) is
interleaved with block i+1's head on each engine's instruction stream.
"""

from contextlib import ExitStack

import numpy as np

import concourse.bass as bass
import concourse.mybir as mybir
from concourse.bass_utils import run_bass_kernel_spmd

F32 = mybir.dt.float32
G = 8
K = 4
ZDIM = 72  # 8 y + 8 groups * 8 max-slots

NCORES = 8
B, C, H, W = 16, 256, 128, 128
NB = B // NCORES  # batches per core

_DELTA_DT = mybir.dt.float16
_DELTA_NP = np.float16
_HBLK = 4


def _build_consts(soft_w1, soft_w2, top_w1, top_w2, r):
    soft_w1 = np.asarray(soft_w1, np.float32)
    soft_w2 = np.asarray(soft_w2, np.float32)
    top_w1 = np.asarray(top_w1, np.float32)
    top_w2 = np.asarray(top_w2, np.float32)
    r = np.asarray(r, np.float32)

    w = np.exp(r - r.max())
    w = w / w.sum()
    rt, rs = np.float32(w[0]), np.float32(w[1])

    w2eff = np.zeros((2, ZDIM, C // 2), np.float32)
    for g in range(G):
        for hf in range(2):
            cols = slice(hf * (C // 2), (hf + 1) * (C // 2))
            w2eff[hf, g, :] = rs * soft_w2[cols, g]
            for k in range(K):
                w2eff[hf, 8 + 8 * g + k, :] = rt * top_w2[cols, g] * top_w1[g, k]
    w2eff = np.ascontiguousarray(w2eff.astype(_DELTA_NP))

    masks = np.zeros((2, 128, 8), np.float32)
    for hf in range(2):
        for j in range(4):
            rows = slice(j * 32, (j + 1) * 32)
            masks[hf, rows, j] = 1.0
            masks[hf, rows, 4 + j] = soft_w1[hf * 4 + j, :]

    ident = np.eye(128, dtype=np.float32)
    return {"w2eff": w2eff, "masks": masks, "ident": ident}


def _build_kernel(NB=NB, NH=H, HBLK=_HBLK, delta_dtype=_DELTA_DT, loops=1):
    assert NH % HBLK == 0 and HBLK == 4
    nc = bass.Bass("TRN2", target_bir_lowering=False, debug=False)

    x_d = nc.dram_tensor("x", [NB, C, NH, W], F32, kind="ExternalInput").ap()
    w2eff_d = nc.dram_tensor("w2eff", [2, ZDIM, 128], delta_dtype,
                             kind="ExternalInput").ap()
    masks_d = nc.dram_tensor("masks", [2, 128, 8], F32, kind="ExternalInput").ap()
    ident_d = nc.dram_tensor("ident", [128, 128], F32, kind="ExternalInput").ap()
    out_d = nc.dram_tensor("out", [NB, C, NH, W], F32, kind="ExternalOutput").ap()

    P = HBLK * 128          # 512 pixels per h-block
    NBLK0 = NB * (NH // HBLK)
    NBLK = NBLK0 * loops
    Exp = mybir.ActivationFunctionType.Exp

    def blk(i):
        i = i % NBLK0
        b = i // (NH // HBLK)
        h0 = (i % (NH // HBLK)) * HBLK
        return b, h0

    with ExitStack() as ctx:
        def sb(name, shape, dtype=F32):
            return ctx.enter_context(nc.sbuf_tensor(name, shape, dtype))

        def ps(name, shape, dtype=F32):
            return ctx.enter_context(nc.psum_tensor(name, shape, dtype))

        def sem(name):
            return ctx.enter_context(nc.semaphore(name))

        # constants
        ident = sb("identc", [128, 128])
        mask0 = sb("mask0", [128, 8])
        mask1 = sb("mask1", [128, 8])
        w2e0 = sb("w2e0", [ZDIM, 128], delta_dtype)
        w2e1 = sb("w2e1", [ZDIM, 128], delta_dtype)

        # ring buffers
        XD, ED, SD, OD = 6, 3, 3, 3
        x0 = [sb(f"x0_{j}", [128, P]) for j in range(XD)]
        x1 = [sb(f"x1_{j}", [128, P]) for j in range(XD)]
        e0 = [sb(f"e0_{j}", [128, P]) for j in range(ED)]
        e1 = [sb(f"e1_{j}", [128, P]) for j in range(ED)]
        xe0 = [sb(f"xe0_{j}", [128, P]) for j in range(ED)]
        xe1 = [sb(f"xe1_{j}", [128, P]) for j in range(ED)]
        xp_sb = [[sb(f"xp_{u}_{j}", [128, 512]) for j in range(2)]
                 for u in range(2)]
        z4 = [sb(f"z4_{j}", [128, HBLK * ZDIM]) for j in range(2)]
        rcp1 = sb("rcp1", [128, HBLK * 8])
        zT_sb = [sb(f"zT_{j}", [ZDIM, P], delta_dtype) for j in range(2)]
        o0 = [sb(f"o0_{j}", [128, P]) for j in range(OD)]
        o1 = [sb(f"o1_{j}", [128, P]) for j in range(OD)]

        # psum (8 banks total)
        xp_ps = [ps(f"xpps_{u}", [128, 512]) for u in range(2)]
        sn_ps = ps("snps", [128, HBLK * 16])
        zT_ps = ps("ztps", [ZDIM, P])
        d0_ps = [ps(f"d0ps_{j}", [128, P]) for j in range(2)]
        d1_ps = [ps(f"d1ps_{j}", [128, P]) for j in range(2)]

        # semaphores
        s_x0 = [sem(f"s_x0_{j}") for j in range(XD)]
        s_x1 = [sem(f"s_x1_{j}") for j in range(XD)]
        s_st0 = [sem(f"s_st0_{j}") for j in range(OD)]
        s_st1 = [sem(f"s_st1_{j}") for j in range(OD)]
        s_cst = sem("s_cst")
        s_exp = sem("s_exp")   # +1 after exp1(i)          -> i+1
        s_xe = sem("s_xe")     # +1 after xe1(i)           -> i+1
        s_xpc = sem("s_xpc")   # +1 after xp-copy(u,i)     -> 2i+u+1
        s_snc = sem("s_snc")   # +1 after sn-copy(i)       -> i+1
        s_ztc = sem("s_ztc")   # +1 after zt-copy(i)       -> i+1
        s_div = sem("s_div")   # +1 after y-mul(i)         -> i+1
        s_rcp = sem("s_rcp")   # +1 after recip(i)         -> i+1
        s_tx = sem("s_tx")     # +1 after T(x) pair-u(i)   -> 2i+u+1
        s_mm = sem("s_mm")     # +1 after mm_sn(i) last    -> i+1
        s_tz = sem("s_tz")     # +1 after T(z)(i) last     -> i+1
        s_dl = sem("s_dl")     # +1 after delta1(i)        -> i+1
        s_mx = sem("s_mx")     # +1 after last max(i)      -> i+1
        s_ad = sem("s_ad")     # +1 after add1(i)          -> i+1

        with nc.Block() as block:

            @block.sync
            def _(sync):
                # constants: one queue (SP hwdge), FIFO
                sync.dma_start(ident[:], ident_d[:]).then_inc(s_cst, 16)
                sync.dma_start(mask0[:], masks_d[0]).then_inc(s_cst, 16)
                sync.dma_start(mask1[:], masks_d[1]).then_inc(s_cst, 16)
                sync.dma_start(w2e0[:], w2eff_d[0]).then_inc(s_cst, 16)
                sync.dma_start(w2e1[:], w2eff_d[1]).then_inc(s_cst, 16)
                for i in range(NBLK + 1):
                    if i < NBLK:
                        b, h0 = blk(i)
                        if i >= XD:
                            sync.wait_ge(s_ad, i - XD + 1)
                        j = i % XD
                        sync.dma_start(
                            x0[j].ap().rearrange("p (h w) -> p h w", h=HBLK),
                            x_d[b, 0:128, h0:h0 + HBLK, :],
                        ).then_inc(s_x0[j], 16)
                        sync.dma_start(
                            x1[j].ap().rearrange("p (h w) -> p h w", h=HBLK),
                            x_d[b, 128:256, h0:h0 + HBLK, :],
                        ).then_inc(s_x1[j], 16)

            @block.scalar
            def _(scalar):
                for i in range(NBLK + 3):
                    j = i % 2
                    if i < NBLK:
                        je = i % ED
                        if i >= ED:
                            scalar.wait_ge(s_xe, i - ED + 1)   # e WAR vs Pool
                            scalar.wait_ge(s_mm, i - ED + 1)   # e WAR vs PE
                        scalar.wait_ge(s_x0[i % XD], 16 * (i // XD + 1))
                        scalar.activation(e0[je][:], x0[i % XD][:], Exp)
                        scalar.wait_ge(s_x1[i % XD], 16 * (i // XD + 1))
                        scalar.activation(e1[je][:], x1[i % XD][:], Exp) \
                            .then_inc(s_exp, 1)
                    if 2 <= i <= NBLK + 1:
                        # z^T copy for block i-2
                        if i >= 4:
                            scalar.wait_ge(s_dl, i - 3)   # zT_sb WAR vs delta
                        scalar.wait_ge(s_tz, i - 1)
                        scalar.copy(zT_sb[(i - 2) % 2][:], zT_ps[:]) \
                            .then_inc(s_ztc, 1)
                    if i < NBLK:
                        if i >= 2:
                            scalar.wait_ge(s_mx, i - 1)   # xp_sb WAR vs maxes
                        for u in range(2):
                            scalar.wait_ge(s_tx, 2 * i + u + 1)
                            scalar.copy(xp_sb[u][j][:], xp_ps[u][:]) \
                                .then_inc(s_xpc, 1)


            @block.gpsimd
            def _(gpsimd):
                for i in range(NBLK + 3):
                    j = i % 2
                    if i < NBLK:
                        je = i % ED
                        if i >= ED:
                            gpsimd.wait_ge(s_mm, i - ED + 1)  # xe WAR vs PE
                        gpsimd.wait_ge(s_exp, i + 1)
                        gpsimd.tensor_mul(xe0[je][:], x0[i % XD][:], e0[je][:])
                        gpsimd.tensor_mul(xe1[je][:], x1[i % XD][:],
                                          e1[je][:]).then_inc(s_xe, 1)
                    if 3 <= i <= NBLK + 2:
                        # stores for block i-3 via SWDGE queues
                        bp, hp = blk(i - 3)
                        gpsimd.wait_ge(s_ad, i - 2)
                        jo = (i - 3) % OD
                        gpsimd.dma_start(
                            out_d[bp, 0:128, hp:hp + HBLK, :],
                            o0[jo].ap().rearrange("p (h w) -> p h w", h=HBLK),
                        ).then_inc(s_st0[jo], 16)
                        gpsimd.dma_start(
                            out_d[bp, 128:256, hp:hp + HBLK, :],
                            o1[jo].ap().rearrange("p (h w) -> p h w", h=HBLK),
                        ).then_inc(s_st1[jo], 16)

            @block.tensor
            def _(tensor):
                tensor.wait_ge(s_cst, 80)
                for i in range(NBLK + 3):
                    if i < NBLK:
                        tensor.wait_ge(s_x0[i % XD], 16 * (i // XD + 1))
                        tensor.wait_ge(s_x1[i % XD], 16 * (i // XD + 1))
                        for u in range(2):
                            if i >= 1:
                                tensor.wait_ge(s_xpc, 2 * i - 1 + u)
                            for v in range(2):
                                t = 2 * u + v
                                px = bass.ts(t, 128)
                                tensor.transpose(
                                    xp_ps[u][:, v * 256:v * 256 + 128],
                                    x0[i % XD][:, px], ident[:])
                                mm = tensor.transpose(
                                    xp_ps[u][:, v * 256 + 128:v * 256 + 256],
                                    x1[i % XD][:, px], ident[:])
                                if v == 1:
                                    mm.then_inc(s_tx, 1)
                        if i >= 1:
                            tensor.wait_ge(s_div, i)      # sn_ps WAR (DVE read)
                        tensor.wait_ge(s_exp, i + 1)
                        tensor.wait_ge(s_xe, i + 1)
                        for t in range(HBLK):
                            px = bass.ts(t, 128)
                            c = t * 16
                            tensor.matmul(sn_ps[:, c + 0:c + 4],
                                          e0[i % ED][:, px],
                                          mask0[:, 0:4], start=True, stop=True)
                            tensor.matmul(sn_ps[:, c + 4:c + 8],
                                          xe0[i % ED][:, px],
                                          mask0[:, 4:8], start=True, stop=True)
                            tensor.matmul(sn_ps[:, c + 8:c + 12],
                                          e1[i % ED][:, px],
                                          mask1[:, 0:4], start=True, stop=True)
                            mm = tensor.matmul(sn_ps[:, c + 12:c + 16],
                                               xe1[i % ED][:, px],
                                               mask1[:, 4:8],
                                               start=True, stop=True)
                            if t == HBLK - 1:
                                mm.then_inc(s_mm, 1)
                    if 2 <= i <= NBLK + 1:
                        # deltas for block i-2
                        jq = (i - 2) % 2
                        if i >= 4:
                            tensor.wait_ge(s_ad, i - 3)   # d_ps WAR
                        tensor.wait_ge(s_ztc, i - 1)
                        tensor.matmul(d0_ps[jq][:], w2e0[:], zT_sb[jq][:],
                                      start=True, stop=True)
                        tensor.matmul(d1_ps[jq][:], w2e1[:], zT_sb[jq][:],
                                      start=True, stop=True).then_inc(s_dl, 1)
                    if 1 <= i <= NBLK:
                        jp = (i - 1) % 2
                        tensor.wait_ge(s_mx, i)
                        tensor.wait_ge(s_div, i)
                        for t in range(HBLK):
                            mm = tensor.transpose(
                                zT_ps[:, t * 128:(t + 1) * 128],
                                z4[jp][:, t * ZDIM:(t + 1) * ZDIM], ident[:])
                            if t == HBLK - 1:
                                mm.then_inc(s_tz, 1)

            @block.vector
            def _(vector):
                for i in range(NBLK + 3):
                    j = i % 2
                    if 1 <= i <= NBLK:
                        # y(i-1) = num(i-1)/s(i-1) straight from PSUM
                        jp = (i - 1) % 2
                        vector.wait_ge(s_mm, i)
                        if i >= 2:
                            vector.wait_ge(s_tz, i - 1)   # z4 WAR (mul + maxes)
                        snp = sn_ps.ap().rearrange(
                            "p (t hf x g) -> p t hf x g", t=HBLK, hf=2, x=2)
                        rcv = rcp1.ap().rearrange(
                            "p (t hf g) -> p t hf g", t=HBLK, hf=2)
                        vector.reciprocal(rcv, snp[:, :, :, 0, :]) \
                            .then_inc(s_rcp, 1)
                        vector.wait_ge(s_rcp, i)
                        z4v = z4[jp].ap().rearrange(
                            "p (t a hf g) -> p t a hf g", t=HBLK, a=9, hf=2)
                        vector.tensor_tensor(
                            z4v[:, :, 0, :, :], snp[:, :, :, 1, :],
                            rcv, op=mybir.AluOpType.mult).then_inc(s_div, 1)
                    if i < NBLK:
                        if i >= 2 and i > NBLK:
                            vector.wait_ge(s_tz, i - 1)   # covered above
                        for u in range(2):
                            vector.wait_ge(s_xpc, 2 * i + u + 1)
                            for v in range(2):
                                t = 2 * u + v
                                for g in range(G):
                                    mx = vector.max(
                                        z4[j][:, t * ZDIM + 8 + 8 * g:
                                              t * ZDIM + 16 + 8 * g],
                                        xp_sb[u][j][:, v * 256 + g * 32:
                                                    v * 256 + (g + 1) * 32])
                        mx.then_inc(s_mx, 1)
                    if 3 <= i <= NBLK + 2:
                        jp = (i - 3) % 2
                        jo = (i - 3) % OD
                        vector.wait_ge(s_dl, i - 2)
                        if i - 3 >= OD:
                            vector.wait_ge(s_st0[jo], 16 * ((i - 3) // OD))
                            vector.wait_ge(s_st1[jo], 16 * ((i - 3) // OD))
                        vector.tensor_add(o0[jo][:], x0[(i - 3) % XD][:],
                                          d0_ps[jp][:])
                        vector.tensor_add(o1[jo][:], x1[(i - 3) % XD][:],
                                          d1_ps[jp][:]).then_inc(s_ad, 1)

    return nc


_NC_CACHE = {}


def _get_nc(loops=1):
    if loops not in _NC_CACHE:
        _NC_CACHE[loops] = _build_kernel(loops=loops)
    return _NC_CACHE[loops]


def kernel(x, soft_w1, soft_w2, top_w1, top_w2, r, _trace=False, _tmpdir=None,
           _loops=1):
    x = np.ascontiguousarray(np.asarray(x, np.float32))
    assert x.shape == (B, C, H, W), x.shape
    consts = _build_consts(soft_w1, soft_w2, top_w1, top_w2, r)

    nc = _get_nc(_loops)
    in_maps = []
    for i in range(NCORES):
        in_maps.append({
            "x": np.ascontiguousarray(x[i * NB:(i + 1) * NB]),
            "w2eff": consts["w2eff"],
            "masks": consts["masks"],
            "ident": consts["ident"],
        })
    res = run_bass_kernel_spmd(nc, in_maps, core_ids=list(range(NCORES)),
                               trace=_trace, tmpdir=_tmpdir)
    out = np.concatenate(
        [np.asarray(res.results[i]["out"]).reshape(NB, C, H, W)
         for i in range(NCORES)], axis=0)
    if _trace:
        return out, res
    return out



# revision 2
# speedup vs baseline: 10.9387x; 10.9387x over previous
"""nn_CGBlock Trainium2 kernel v2: f16 I/O, dual-layout loads, DVE-lean.

Data-parallel over batch: 8 NeuronCores x 2 batches each.

Host packs per h-block (4 rows x 128 w = 512 px) one [128, 2048] f16 tile:
  cols    0: 512  x0  ch-major (partition=c 0..127,  free=(t,w))
  cols  512:1024  x1  ch-major (partition=c-128,     free=(t,w))
  cols 1024:2048  xpx px-major (partition=w,         free=(t,c))
Output per block: [128, 1024] f16 (o0 | o1 ch-major), host un-permutes.

Per-block engine split (steady state ~3.2us):
  SP  : 1 HWDGE load (4KB/part), 1 HWDGE store (2KB/part)
  ACT : exp0, exp1 (f16 2x); out0/out1 PSUM->f16 copies; num copy; zT copy
  DVE : recip(prev); 32x max8 (top-8 per (tile,group) window)  <- bottleneck
  Pool: xe0, xe1 = x*e; ymul y = num * (1/s) into z4 y-slots
  PE  : x-accum (ident matmul, start) + delta matmul (stop) -> d = x + delta;
        16 tiny sn matmuls (s, num per (px,group)); 4 z4->zT transposes
"""

from contextlib import ExitStack

import numpy as np

import concourse.bass as bass
import concourse.mybir as mybir
from concourse.bass_utils import run_bass_kernel_spmd

F32 = mybir.dt.float32
F16 = mybir.dt.float16
NPF16 = np.float16

G = 8
K = 4
ZDIM = 72  # 8 y + 8 groups * 8 max-slots

NCORES = 8
B, C, H, W = 16, 256, 128, 128
NB = B // NCORES
HBLK = 4
NBLK0 = NB * (H // HBLK)  # 64 blocks per core
P = HBLK * W              # 512 px per block

XD = 6   # xall ring
OD = 3   # oall ring


def _build_consts(soft_w1, soft_w2, top_w1, top_w2, r):
    soft_w1 = np.asarray(soft_w1, np.float32)
    soft_w2 = np.asarray(soft_w2, np.float32)
    top_w1 = np.asarray(top_w1, np.float32)
    top_w2 = np.asarray(top_w2, np.float32)
    r = np.asarray(r, np.float32)

    w = np.exp(r - r.max())
    w = w / w.sum()
    rt, rs = np.float32(w[0]), np.float32(w[1])

    w2eff = np.zeros((2, ZDIM, C // 2), np.float32)
    for g in range(G):
        for hf in range(2):
            cols = slice(hf * (C // 2), (hf + 1) * (C // 2))
            w2eff[hf, g, :] = rs * soft_w2[cols, g]
            for k in range(K):
                w2eff[hf, 8 + 8 * g + k, :] = rt * top_w2[cols, g] * top_w1[g, k]
    w2eff = np.ascontiguousarray(w2eff.astype(NPF16))

    masks = np.zeros((2, 128, 8), np.float32)
    for hf in range(2):
        for j in range(4):
            rows = slice(j * 32, (j + 1) * 32)
            masks[hf, rows, j] = 1.0
            masks[hf, rows, 4 + j] = soft_w1[hf * 4 + j, :]
    masks = np.ascontiguousarray(masks.astype(NPF16))

    ident = np.eye(128, dtype=NPF16)
    return {"w2eff": w2eff, "masks": masks, "ident": ident}


def _prep_x(x_core):
    """[NB, C, H, W] f32 -> [NBLK0, 128, 2048] f16 (x0 | x1 | xpx)."""
    nb = x_core.shape[0]
    nblk = nb * (x_core.shape[2] // HBLK)
    xh = np.asarray(x_core, NPF16)
    A = xh.reshape(nb, C, -1, HBLK, W)            # b, c, hb, t, w
    X = np.empty((nb, A.shape[2], 128, 2048), NPF16)
    x01 = A.transpose(0, 2, 1, 3, 4).reshape(nb, A.shape[2], C, HBLK * W)
    X[..., 0:512] = x01[:, :, 0:128, :]
    X[..., 512:1024] = x01[:, :, 128:256, :]
    X[..., 1024:2048] = A.transpose(0, 2, 4, 3, 1).reshape(
        nb, A.shape[2], W, HBLK * C)
    return np.ascontiguousarray(X.reshape(nblk, 128, 2048))


def _unprep_out(o_all, nb=NB, nh=H):
    """[NBLK0, 128, 1024] f16 -> [nb, C, nh, W] f32."""
    nhb = nh // HBLK
    O = o_all.reshape(nb, nhb, 128, 2, HBLK, W)    # b, hb, c, half, t, w
    O = O.transpose(0, 3, 2, 1, 4, 5)              # b, half, c, hb, t, w
    return np.ascontiguousarray(O.astype(np.float32).reshape(nb, C, nh, W))


def _build_kernel(NBLKC=NBLK0, loops=1):
    nc = bass.Bass("TRN2", target_bir_lowering=False, debug=False)

    x_d = nc.dram_tensor("xin", [NBLKC, 128, 2048], F16,
                         kind="ExternalInput").ap()
    w2eff_d = nc.dram_tensor("w2eff", [2, ZDIM, 128], F16,
                             kind="ExternalInput").ap()
    masks_d = nc.dram_tensor("masks", [2, 128, 8], F16,
                             kind="ExternalInput").ap()
    ident_d = nc.dram_tensor("ident", [128, 128], F16,
                             kind="ExternalInput").ap()
    out_d = nc.dram_tensor("out", [NBLKC, 128, 1024], F16,
                           kind="ExternalOutput").ap()

    NBLK = NBLKC * loops
    Exp = mybir.ActivationFunctionType.Exp

    def blk(i):
        return i % NBLKC

    with ExitStack() as ctx:
        def sb(name, shape, dtype=F32):
            return ctx.enter_context(nc.sbuf_tensor(name, shape, dtype))

        def ps(name, shape, dtype=F32):
            return ctx.enter_context(nc.psum_tensor(name, shape, dtype))

        def sem(name):
            return ctx.enter_context(nc.semaphore(name))

        # constants
        identf = sb("identc", [128, 128], F16)
        mask0 = sb("mask0", [128, 8], F16)
        mask1 = sb("mask1", [128, 8], F16)
        w2e0 = sb("w2e0", [ZDIM, 128], F16)
        w2e1 = sb("w2e1", [ZDIM, 128], F16)

        # rings
        xall = [sb(f"xall{j}", [128, 2048], F16) for j in range(XD)]
        e0 = [sb(f"e0_{j}", [128, P], F16) for j in range(2)]
        e1 = [sb(f"e1_{j}", [128, P], F16) for j in range(2)]
        xe0 = [sb(f"xe0_{j}", [128, P], F16) for j in range(2)]
        xe1 = [sb(f"xe1_{j}", [128, P], F16) for j in range(2)]
        z4 = [sb(f"z4_{j}", [128, HBLK * ZDIM], F16) for j in range(3)]
        num_sb = [sb(f"num_{j}", [128, 32]) for j in range(2)]
        rcp1 = [sb(f"rcp_{j}", [128, 32]) for j in range(2)]
        zT_sb = [sb(f"zT_{j}", [ZDIM, P], F16) for j in range(2)]
        oall = [sb(f"oall{j}", [128, 1024], F16) for j in range(OD)]

        # psum (8 banks)
        sn_ps = [ps(f"snps{j}", [128, HBLK * 16]) for j in range(2)]
        zT_ps = [ps(f"ztps{j}", [ZDIM, P], F16) for j in range(2)]
        d0_ps = [ps(f"d0ps{j}", [128, P]) for j in range(2)]
        d1_ps = [ps(f"d1ps{j}", [128, P]) for j in range(2)]

        # semaphores
        s_x = [sem(f"s_x{j}") for j in range(XD)]
        s_st = [sem(f"s_st{j}") for j in range(OD)]
        s_cst = sem("s_cst")
        s_exp = sem("s_exp")   # +1 per exp half       -> 2i+2 after block i
        s_xe = sem("s_xe")     # +1 per xe half        -> 2i+2
        s_sn = sem("s_sn")     # +1 after last sn mm   -> i+1
        s_rc = sem("s_rc")     # +1 after recip(i)     -> i+1
        s_nc = sem("s_nc")     # +1 after num copy(i)  -> i+1
        s_ym = sem("s_ym")     # +1 after ymul(i)      -> i+1
        s_mx = sem("s_mx")     # +1 after last max8(i) -> i+1
        s_tz = sem("s_tz")     # +1 after last T(z)(i) -> i+1
        s_ztc = sem("s_ztc")   # +1 after zT copy(i)   -> i+1
        s_dl = sem("s_dl")     # +1 after delta1(i)    -> i+1
        s_oc = sem("s_oc")     # +1 after out1 copy(i) -> i+1
        s_xa = sem("s_xa")     # +1 after xacc pair(i) -> i+1

        def snp(i):
            return sn_ps[i % 2].ap().rearrange(
                "p (t hf x g) -> p t hf x g", t=HBLK, hf=2, x=2)

        def rcv(i):
            return rcp1[i % 2].ap().rearrange(
                "p (t hf g) -> p t hf g", t=HBLK, hf=2)

        def numv(i):
            return num_sb[i % 2].ap().rearrange(
                "p (t hf g) -> p t hf g", t=HBLK, hf=2)

        def z4y(i):
            return z4[i % 3].ap().rearrange(
                "p (t a hf g) -> p t a hf g", t=HBLK, a=9, hf=2)[:, :, 0, :, :]

        with nc.Block() as block:

            @block.sync
            def _(sync):
                sync.dma_start(identf[:], ident_d[:]).then_inc(s_cst, 16)
                sync.dma_start(mask0[:], masks_d[0]).then_inc(s_cst, 16)
                sync.dma_start(mask1[:], masks_d[1]).then_inc(s_cst, 16)
                sync.dma_start(w2e0[:], w2eff_d[0]).then_inc(s_cst, 16)
                sync.dma_start(w2e1[:], w2eff_d[1]).then_inc(s_cst, 16)
                for i in range(NBLK + 5):
                    p = i - 4
                    if 0 <= p < NBLK:
                        sync.wait_ge(s_oc, p + 1)
                        sync.dma_start(out_d[blk(p)], oall[p % OD][:]) \
                            .then_inc(s_st[p % OD], 16)
                    js = [0, 1, 2] if i == 0 else [i + 2]
                    for j in js:
                        if not (0 <= j < NBLK) or (i > 0 and j < 3):
                            continue
                        if j >= XD:
                            sync.wait_ge(s_xa, j - XD + 1)
                        sync.dma_start(xall[j % XD][:], x_d[blk(j)]) \
                            .then_inc(s_x[j % XD], 16)

            @block.scalar
            def _(scalar):
                for i in range(NBLK + 3):
                    if i < NBLK:
                        scalar.wait_ge(s_x[i % XD], 16 * (i // XD + 1))
                        if i >= 2:
                            scalar.wait_ge(s_sn, i - 1)       # e WAR (PE)
                            scalar.wait_ge(s_xe, 2 * (i - 1))  # e WAR (Pool)
                        scalar.activation(e0[i % 2][:], xall[i % XD][:, 0:512],
                                          Exp).then_inc(s_exp, 1)
                        scalar.activation(e1[i % 2][:],
                                          xall[i % XD][:, 512:1024],
                                          Exp).then_inc(s_exp, 1)
                    p = i - 3
                    if 0 <= p < NBLK:
                        scalar.wait_ge(s_dl, p + 1)
                        if p >= OD:
                            scalar.wait_ge(s_st[p % OD],
                                           16 * ((p - OD) // OD + 1))
                        scalar.copy(oall[p % OD][:, 0:512], d0_ps[p % 2][:])
                        scalar.copy(oall[p % OD][:, 512:1024],
                                    d1_ps[p % 2][:]).then_inc(s_oc, 1)
                    q = i - 1
                    if 0 <= q < NBLK:
                        scalar.wait_ge(s_sn, q + 1)
                        if q >= 2:
                            scalar.wait_ge(s_ym, q - 1)       # num_sb WAR
                        scalar.copy(numv(q), snp(q)[:, :, :, 1, :]) \
                            .then_inc(s_nc, 1)
                    r = i - 2
                    if 0 <= r < NBLK:
                        scalar.wait_ge(s_tz, r + 1)
                        if r >= 2:
                            scalar.wait_ge(s_dl, r - 1)       # zT_sb WAR
                        scalar.copy(zT_sb[r % 2][:], zT_ps[r % 2][:]) \
                            .then_inc(s_ztc, 1)

            @block.vector
            def _(vector):
                for i in range(NBLK + 1):
                    q = i - 1
                    if 0 <= q < NBLK:
                        vector.wait_ge(s_sn, q + 1)
                        if q >= 2:
                            vector.wait_ge(s_ym, q - 1)       # rcp WAR
                        vector.reciprocal(rcv(q), snp(q)[:, :, :, 0, :]) \
                            .then_inc(s_rc, 1)
                    if i < NBLK:
                        vector.wait_ge(s_x[i % XD], 16 * (i // XD + 1))
                        if i >= 3:
                            vector.wait_ge(s_tz, i - 2)       # z4 WAR
                        for t in range(HBLK):
                            for g in range(G):
                                mx = vector.max(
                                    z4[i % 3][:, t * ZDIM + 8 + 8 * g:
                                              t * ZDIM + 16 + 8 * g],
                                    xall[i % XD][:, 1024 + t * 256 + g * 32:
                                                 1024 + t * 256 + g * 32 + 32])
                        mx.then_inc(s_mx, 1)

            @block.gpsimd
            def _(gpsimd):
                for i in range(NBLK + 1):
                    if i < NBLK:
                        if i >= 2:
                            gpsimd.wait_ge(s_sn, i - 1)       # xe WAR (PE)
                        gpsimd.wait_ge(s_exp, 2 * i + 1)
                        gpsimd.tensor_mul(xe0[i % 2][:], xall[i % XD][:, 0:512],
                                          e0[i % 2][:]).then_inc(s_xe, 1)
                        gpsimd.wait_ge(s_exp, 2 * i + 2)
                        gpsimd.tensor_mul(xe1[i % 2][:],
                                          xall[i % XD][:, 512:1024],
                                          e1[i % 2][:]).then_inc(s_xe, 1)
                    q = i - 1
                    if 0 <= q < NBLK:
                        gpsimd.wait_ge(s_nc, q + 1)
                        gpsimd.wait_ge(s_rc, q + 1)
                        if q >= 3:
                            gpsimd.wait_ge(s_tz, q - 2)       # z4 WAR
                        gpsimd.tensor_mul(z4y(q), numv(q), rcv(q)) \
                            .then_inc(s_ym, 1)

            @block.tensor
            def _(tensor):
                tensor.wait_ge(s_cst, 80)
                for i in range(NBLK + 2):
                    p = i - 2
                    if 0 <= p < NBLK:
                        if p >= 2:
                            tensor.wait_ge(s_oc, p - 1)       # d_ps WAR
                        tensor.matmul(d0_ps[p % 2][:], identf[:],
                                      xall[p % XD][:, 0:512],
                                      start=True, stop=False)
                        tensor.matmul(d1_ps[p % 2][:], identf[:],
                                      xall[p % XD][:, 512:1024],
                                      start=True, stop=False).then_inc(s_xa, 1)
                    if i < NBLK:
                        if i >= 2:
                            tensor.wait_ge(s_rc, i - 1)       # sn_ps WAR
                            tensor.wait_ge(s_nc, i - 1)
                        tensor.wait_ge(s_exp, 2 * i + 1)
                        for t in range(HBLK):
                            tensor.matmul(
                                sn_ps[i % 2][:, 16 * t:16 * t + 4],
                                e0[i % 2][:, 128 * t:128 * t + 128],
                                mask0[:, 0:4], start=True, stop=True)
                        tensor.wait_ge(s_exp, 2 * i + 2)
                        for t in range(HBLK):
                            tensor.matmul(
                                sn_ps[i % 2][:, 16 * t + 8:16 * t + 12],
                                e1[i % 2][:, 128 * t:128 * t + 128],
                                mask1[:, 0:4], start=True, stop=True)
                        tensor.wait_ge(s_xe, 2 * i + 1)
                        for t in range(HBLK):
                            tensor.matmul(
                                sn_ps[i % 2][:, 16 * t + 4:16 * t + 8],
                                xe0[i % 2][:, 128 * t:128 * t + 128],
                                mask0[:, 4:8], start=True, stop=True)
                        tensor.wait_ge(s_xe, 2 * i + 2)
                        for t in range(HBLK):
                            mm = tensor.matmul(
                                sn_ps[i % 2][:, 16 * t + 12:16 * t + 16],
                                xe1[i % 2][:, 128 * t:128 * t + 128],
                                mask1[:, 4:8], start=True, stop=True)
                        mm.then_inc(s_sn, 1)
                    if 0 <= p < NBLK:
                        tensor.wait_ge(s_ztc, p + 1)
                        tensor.matmul(d0_ps[p % 2][:], w2e0[:], zT_sb[p % 2][:],
                                      start=False, stop=True)
                        tensor.matmul(d1_ps[p % 2][:], w2e1[:], zT_sb[p % 2][:],
                                      start=False, stop=True).then_inc(s_dl, 1)
                    r = i - 1
                    if 0 <= r < NBLK:
                        if r >= 2:
                            tensor.wait_ge(s_ztc, r - 1)      # zT_ps WAR
                        tensor.wait_ge(s_ym, r + 1)
                        tensor.wait_ge(s_mx, r + 1)
                        for t in range(HBLK):
                            mm = tensor.transpose(
                                zT_ps[r % 2][:, 128 * t:128 * t + 128],
                                z4[r % 3][:, ZDIM * t:ZDIM * t + ZDIM],
                                identf[:])
                        mm.then_inc(s_tz, 1)

    return nc


_NC_CACHE = {}


def _get_nc(loops=1):
    if loops not in _NC_CACHE:
        _NC_CACHE[loops] = _build_kernel(loops=loops)
    return _NC_CACHE[loops]


def _make_in_maps(x, soft_w1, soft_w2, top_w1, top_w2, r):
    x = np.asarray(x, np.float32)
    consts = _build_consts(soft_w1, soft_w2, top_w1, top_w2, r)
    in_maps = []
    for i in range(NCORES):
        in_maps.append({
            "xin": _prep_x(x[i * NB:(i + 1) * NB]),
            "w2eff": consts["w2eff"],
            "masks": consts["masks"],
            "ident": consts["ident"],
        })
    return in_maps


def kernel(x, soft_w1, soft_w2, top_w1, top_w2, r, _trace=False, _tmpdir=None,
           _loops=1):
    assert np.asarray(x).shape == (B, C, H, W)
    in_maps = _make_in_maps(x, soft_w1, soft_w2, top_w1, top_w2, r)
    nc = _get_nc(_loops)
    res = run_bass_kernel_spmd(nc, in_maps, core_ids=list(range(NCORES)),
                               trace=_trace, tmpdir=_tmpdir)
    out = np.concatenate(
        [_unprep_out(np.asarray(res.results[i]["out"]).reshape(NBLK0, 128,
                                                               1024))
         for i in range(NCORES)], axis=0)
    if _trace:
        return out, res
    return out
